# revision 22
# baseline (speedup 1.0000x reference)
"""LIF Conv2d + STDP kernel for 8 Trainium2 NeuronCores — v3.

Data-parallel over batch (B=8, one element per core); per-step STDP
weight-gradient correlations AllGathered (bf16) and summed locally.

Changes vs v2:
  - Host-precomputed input layouts: Sc [T,112,2176] bf16 (the S3c
    kw-shifted layout, DMA'd straight in with large bursts) and
    Sl [T,128,32,48] bf16 (l-major S patches), killing the 32 per-step
    PE transposes and the S3cb cast.
  - Outputs written in the native fold layout [T,128,1024] (one DMA
    per tensor per step, 1-4KB bursts); unscrambled on host.
  - Triple-buffered Sc/Sl with 2-step prefetch; double-buffered
    xr/XHf/x3f so the x-path for step t+1 runs during step t's
    AllGather window.
  - The f32r xl correction pass and the Wfix fix-up pass are merged
    into ONE f32r late pass: lhsT = [Wfix ; Wrs_stale] (96 rows), rhs
    XHf = [xr ; xl] stacked per l-half (filled via SBUF-SBUF DMA since
    engine APs must start at partition 0/32/64/96).  24 early + 24
    late f32r matmuls (was 72), telescoping to W_new@x exactly like
    the baseline's three passes.
  - DMA issue spread across engines: prefetch/outputs on sync, x/W
    staging moves on scalar, dw8/cc_in/collective on gpsimd.

Remaining bottleneck: the per-step AllGather round-trip chain
(pack -> trigger ~2.5us -> AG ~7.5us -> dw8 load ~3.8us -> DVE W chain
~3.7us) stalls the PE ~11-19us/step; early conv of t+1 is the only
AG-independent PE work and fills ~8us.  cc payload is column-packed
[48,192] (dWp|dWm) so the consumer needs no cross-partition copy;
dw8 load issues from sync; output DMAs issue after the AG trigger to
keep queues clear for the ring.  cc_in issue must stay on gpsimd
(moving it to sync measured slower).  2-deep pipelining does NOT help:
there is exactly one early-conv block per step to fill each stall.

Known constraints found the hard way: f32r matmuls may only write
PSUM at partition base 0 (bf16 can use 64); tensor_tensor requires
matching start partitions across out/in APs (stt only across inputs);
f32r streams at ~0.83 ns/col regardless of grouping or p-state games
(PE warmer matmuls were tried twice and always run at the 1.2GHz
rate, costing more than they save); moving the Wrs refresh after the
LIF in DVE program order delays the next step's early conv.
"""

import numpy as np

T, B, C_IN, H, W_IN = 32, 8, 16, 64, 64
C_OUT, KH, KW = 32, 3, 3
L = H * W_IN  # 4096
XW = 2176  # 2048 + 2*64 halo
BETA_M = float(np.exp(-1.0 / 20.0))
BETA_S = float(np.exp(-1.0 / 5.0))
BETA_PRE = float(np.exp(-1.0 / 20.0))
BETA_POST = float(np.exp(-1.0 / 20.0))
V_TH = 1.0
T_REF = 2.0
ETA = 5e-4
NORM = float(B * L)
N_CORES = 8


def _patch_tile_drain():
    """walrus in this build rejects >1 sync wait on a CTRL-class (drain)
    instruction; spread the final tile drain's waits across nops."""
    import concourse.tile as tile
    import concourse.mybir as mybir
    from concourse.vector_clock import ScopedClock

    if getattr(tile.TileContext, "_drain_patched", False):
        return

    def _drain_and_barrier(self, tick_clock, wait_clock):
        nc = self.nc
        drain_inst = nc.sync.drain()
        wait_clock.add_sem_waits(
            drain_inst.ins, ScopedClock({None: tick_clock.global_clock})
        )
        si = drain_inst.ins.sync_info
        waits = list(si.on_wait or [])
        if len(waits) > 1:
            si.on_wait = waits[:1]
            for i in range(1, len(waits)):
                nop = nc.sync.nop(nofuse=True)
                nop.ins.sync_info = mybir.SyncInfo(
                    on_wait=waits[i : i + 1], on_update=[]
                )
        nc.all_engine_barrier()
        assert self.sems is not None
        popped = nc._tile_sem_poison_stack.pop()
        assert popped is self._sem_poison
        nc.clear_and_free_semaphores(list(self.sems.allocated().values()))
        nc.all_engine_barrier()

    tile.TileContext._drain_and_barrier = _drain_and_barrier
    tile.TileContext._drain_patched = True


def _split_sync_waits(nc):
    """This walrus build accepts only ONE sync-wait slot per instruction.
    Move extra waits onto injected same-engine nops placed just before."""
    import concourse.mybir as mybir

    n = 0
    for f in nc.m.functions:
        for bb in f.blocks:
            new_insts = []
            for inst in bb.instructions:
                si = inst.sync_info
                waits = list(si.on_wait or []) if si else []
                if len(waits) > 1:
                    for w in waits[:-1]:
                        n += 1
                        nop = mybir.InstNoOp(
                            name=f"I-wsplit-{n}", engine=inst.engine,
                            ins=[], outs=[], bass_nofuse=True,
                            sync_info=mybir.SyncInfo(on_wait=[w], on_update=[]),
                        )
                        new_insts.append(nop)
                    si.on_wait = waits[-1:]
                new_insts.append(inst)
            bb.instructions = new_insts
    return n


_NC_CACHE = {}


def _build(n_steps):
    import concourse.bass as bass
    import concourse.mybir as mybir
    import concourse.tile as tile

    _patch_tile_drain()
    f32 = mybir.dt.float32
    f32r = mybir.dt.float32r
    bf16 = mybir.dt.bfloat16
    u8 = mybir.dt.uint8
    op = mybir.AluOpType

    nc = bass.Bass("TRN2", target_bir_lowering=False, debug=False,
                   num_devices=N_CORES)

    Sc_d = nc.dram_tensor("Sc", [n_steps, 112, XW], bf16, kind="ExternalInput")
    Sl_d = nc.dram_tensor("Sl", [n_steps, 128, 32, 48], bf16,
                          kind="ExternalInput")
    W_d = nc.dram_tensor("Wk", [48, 96], f32, kind="ExternalInput")
    spk_d = nc.dram_tensor("spk_out", [n_steps, 128, 1024], u8,
                           kind="ExternalOutput")
    v_d = nc.dram_tensor("v_out", [n_steps, 128, 1024], f32,
                         kind="ExternalOutput")
    i_d = nc.dram_tensor("i_out", [n_steps, 128, 1024], f32,
                         kind="ExternalOutput")

    cc_in = [nc.dram_tensor(f"cc_in_{t}", [48, 192], bf16)
             for t in range(n_steps - 1)]
    cc_out = [
        nc.dram_tensor(f"cc_out_{t}", [N_CORES, 48, 192], bf16,
                       addr_space="Shared")
        for t in range(n_steps - 1)
    ]

    id128 = nc.inline_tensor(np.eye(128, dtype=np.float32), "id128")

    eta_n = ETA / NORM

    with tile.TileContext(nc) as tc:
        with (
            tc.tile_pool(name="state", bufs=1) as st,
            tc.tile_pool(name="io", bufs=2) as io,
            tc.tile_pool(name="psc", bufs=4, space=bass.MemorySpace.PSUM) as psc,
            tc.tile_pool(name="pst", bufs=3, space=bass.MemorySpace.PSUM) as pst,
            tc.tile_pool(name="psw", bufs=1, space=bass.MemorySpace.PSUM) as psw,
        ):
            # ---------------- persistent state ----------------
            x3f = [st.tile([112, XW], f32, tag=f"x3f{i}", name=f"x3f{i}") for i in range(2)]
            xr = [st.tile([112, XW], f32r, tag=f"xr{i}", name=f"xr{i}") for i in range(2)]
            # XHf[h][buf] (f32r): rows 0-47 = xr, rows 48-95 = xl for
            # l-half h.  WLr rows 0-47 = Wfix (engine-written, critical),
            # rows 48-95 = Wrs_stale (DMA'd off-critical).  The late pass
            # [Wfix;Wrs] @ [xr;xl] telescopes to W_new@x exactly like the
            # baseline's two separate correction passes.
            XHf = [[st.tile([96, XW], f32r, tag=f"XHf{h}{i}", name=f"XHf{h}{i}")
                    for i in range(2)] for h in range(2)]
            Sc = [st.tile([112, XW], bf16, tag=f"Sc{i}", name=f"Sct{i}") for i in range(3)]
            Sl = [st.tile([128, 32, 48], bf16, tag=f"Sl{i}", name=f"Slt{i}") for i in range(3)]
            Wk = st.tile([48, 96], f32, tag="Wk")
            Wrs = st.tile([112, 96], f32r, tag="Wrs")
            WLr = st.tile([96, 96], f32r, tag="WLr")
            wtmp = st.tile([48, 96], f32, tag="wtmp")
            wtmp2 = st.tile([48, 96], f32, tag="wtmp2")
            zero96 = st.tile([48, 96], f32, tag="zero96")
            v = st.tile([128, 1024], f32, tag="v")
            ref = st.tile([128, 1024], f32, tag="ref")
            isyn2 = st.tile([128, 1024], f32, tag="isyn2")
            isyn = st.tile([128, 1024], f32, tag="isyn")
            xlf = st.tile([112, XW], f32r, tag="xlf")
            P3 = [st.tile([128, 32, 48], bf16, tag=f"P3{i}", name=f"P3t{i}") for i in range(2)]
            YIs = st.tile([128, 65, 32], bf16, tag="YIs")
            YIp = [st.tile([128, 65, 32], bf16, tag=f"YIp{i}", name=f"YIpt{i}") for i in range(2)]
            i128b = st.tile([128, 128], bf16, tag="i128b")
            negone = st.tile([128, 1], f32, tag="negone")
            i128f = st.tile([128, 128], f32, tag="i128f")

            # ---------------- prologue ----------------
            nc.sync.dma_start(Wk[:], W_d[:])
            nc.sync.dma_start(i128f[:], id128[:])
            nc.vector.tensor_copy(i128b[:], i128f[:])
            nc.sync.dma_start(Sc[0][:], Sc_d[0])
            nc.sync.dma_start(Sl[0][:], Sl_d[0])
            nc.sync.dma_start(Sc[1][:], Sc_d[1])
            nc.sync.dma_start(Sl[1][:], Sl_d[1])
            nc.vector.memset(x3f[1][:], 0.0)
            nc.vector.memset(zero96[:], 0.0)
            nc.vector.memset(v[:], 0.0)
            nc.vector.memset(ref[:], 0.0)
            nc.vector.memset(P3[1][:], 0.0)
            nc.vector.memset(YIs[:], 0.0)
            nc.vector.memset(YIp[1][:], 0.0)
            nc.vector.memset(negone[:], -1.0)
            nc.vector.scalar_tensor_tensor(
                Wrs[0:48, :], Wk[:], 1.0, zero96[:], op.mult, op.add)
            nc.vector.scalar_tensor_tensor(
                Wrs[64:112, :], Wk[:], 1.0, zero96[:], op.mult, op.add)
            nc.scalar.dma_start(WLr[48:96, :], Wrs[0:48, :])
            # x-path for t=0
            nc.vector.scalar_tensor_tensor(
                x3f[0][:], x3f[1][:], BETA_S, Sc[0][:], op.mult, op.add)
            nc.vector.scalar_tensor_tensor(
                xr[0][:], x3f[1][:], BETA_S, Sc[0][:], op.mult, op.add)
            nc.vector.tensor_tensor(xlf[:], x3f[0][:], xr[0][:], op.subtract)
            for h in range(2):
                r0 = 64 * h
                nc.scalar.dma_start(XHf[h][0][0:48, :], xr[0][r0:r0 + 48, :])
                nc.scalar.dma_start(XHf[h][0][48:96, :], xlf[r0:r0 + 48, :])
            # P3 for t=0
            nc.vector.scalar_tensor_tensor(
                P3[0][:], P3[1][:], BETA_PRE, Sl[0][:], op.mult, op.add)

            for t in range(n_steps):
                tb = t % 2
                nb = (t + 1) % 2
                # ---- 1: prefetch S layouts two steps ahead ----
                if t + 2 < n_steps:
                    nc.sync.dma_start(Sc[(t + 2) % 3][:], Sc_d[t + 2])
                    nc.sync.dma_start(Sl[(t + 2) % 3][:], Sl_d[t + 2])

                # ---- 2: EARLY conv (f32r): isyn = Wrs @ xr ----
                for j in range(8):
                    h, jq = j // 4, j % 4
                    g, d = j % 4, j // 4
                    pe = psc.tile([32, 512], f32, tag="pe")
                    for kh in (1, 0, 2):
                        c0 = 512 * jq + 64 * kh
                        khr = 2 - kh
                        nc.tensor.matmul(
                            pe[:],
                            Wrs[64 * h:64 * h + 48, 32 * khr:32 * khr + 32],
                            xr[tb][64 * h:64 * h + 48, c0:c0 + 512],
                            start=(kh == 1), stop=(kh == 2))
                    sl = isyn[32 * g:32 * g + 32, 512 * d:512 * d + 512]
                    if d == 0:
                        nc.vector.tensor_copy(sl, pe[:])
                    else:
                        nc.scalar.copy(sl, pe[:])

                # ---- 3/4: x-path + P3 for t+1 (fills the AG window) ----
                if t + 1 < n_steps:
                    scn = Sc[(t + 1) % 3]
                    nc.vector.scalar_tensor_tensor(
                        x3f[nb][:], x3f[tb][:], BETA_S, scn[:], op.mult, op.add)
                    nc.vector.scalar_tensor_tensor(
                        xr[nb][:], x3f[tb][:], BETA_S, scn[:], op.mult, op.add)
                    nc.vector.tensor_tensor(xlf[:], x3f[nb][:], xr[nb][:],
                                            op.subtract)
                    for h in range(2):
                        r0 = 64 * h
                        nc.scalar.dma_start(XHf[h][nb][0:48, :],
                                            xr[nb][r0:r0 + 48, :])
                        nc.scalar.dma_start(XHf[h][nb][48:96, :],
                                            xlf[r0:r0 + 48, :])
                    nc.vector.scalar_tensor_tensor(
                        P3[nb][:], P3[tb][:], BETA_PRE, Sl[(t + 1) % 3][:],
                        op.mult, op.add)

                # ---- 5: weight update from AG(t-1) ----
                if t > 0:
                    dw8 = io.tile([48, N_CORES, 192], bf16, tag="dw8")
                    gv = cc_out[t - 1].ap().rearrange("r p c -> p r c")
                    nc.sync.dma_start(dw8[:], gv)
                    s4 = io.tile([48, 4, 192], bf16, tag="s4")
                    nc.vector.tensor_tensor(
                        s4[:], dw8[:, 0:4, :], dw8[:, 4:8, :], op.add)
                    s2 = io.tile([48, 2, 192], bf16, tag="s2")
                    nc.vector.tensor_tensor(
                        s2[:], s4[:, 0:2, :], s4[:, 2:4, :], op.add)
                    dws = io.tile([48, 192], f32, tag="dws")
                    nc.vector.tensor_tensor(
                        dws[:], s2[:, 0, :], s2[:, 1, :], op.add)
                    # fused update: W = W*(1 - eta*(dWp+dWm)) + eta*dWp; clip
                    nc.vector.tensor_tensor(
                        wtmp[:], dws[:, 0:96], dws[:, 96:192], op.add)
                    nc.vector.tensor_scalar(wtmp[:], wtmp[:], -eta_n, 1.0,
                                            op.mult, op.add)
                    nc.vector.tensor_tensor(wtmp2[:], Wk[:], wtmp[:], op.mult)
                    nc.vector.scalar_tensor_tensor(
                        Wk[:], dws[:, 0:96], eta_n, wtmp2[:], op.mult, op.add)
                    nc.vector.tensor_scalar(Wk[:], Wk[:], 1.0, 0.0,
                                            op.min, op.max)

                # ---- 6: Wfix = W_new - Wrs_stale (f32r) ----
                nc.vector.tensor_tensor(wtmp[:], Wk[:], Wrs[0:48, :],
                                        op.subtract)
                nc.vector.scalar_tensor_tensor(
                    WLr[0:48, :], wtmp[:], 1.0, zero96[:], op.mult, op.add)

                # ---- 7: LATE conv + LIF half-interleaved: LIF for half
                # d runs on the DVE while the other half's matmuls stream --
                v1 = io.tile([128, 1024], f32, tag="v1")
                v2 = io.tile([128, 1024], f32, tag="v2")
                spk_bA = io.tile([128, 512], bf16, tag="spk_bA")
                spk_bB = io.tile([128, 512], bf16, tag="spk_bB")
                spk_f = io.tile([128, 1024], f32, tag="spk_f")
                for d in range(2):
                    h = d
                    for jq in range(4):
                        g = jq
                        pl = psc.tile([32, 512], f32, tag="pe", name="pl")
                        for kh in (1, 0, 2):
                            c0 = 512 * jq + 64 * kh
                            khr = 2 - kh
                            nc.tensor.matmul(
                                pl[:],
                                WLr[0:96, 32 * khr:32 * khr + 32],
                                XHf[h][tb][0:96, c0:c0 + 512],
                                start=(kh == 1), stop=(kh == 2))
                        sl = isyn[32 * g:32 * g + 32,
                                  512 * d:512 * d + 512]
                        sl2 = isyn2[32 * g:32 * g + 32,
                                    512 * d:512 * d + 512]
                        nc.vector.tensor_tensor(sl2, sl, pl[:], op.add)
                    spk_h = spk_bA if d == 0 else spk_bB
                    cs = slice(512 * d, 512 * d + 512)
                    nc.vector.scalar_tensor_tensor(
                        v1[:, cs], v[:, cs], BETA_M, isyn2[:, cs],
                        op.mult, op.add)
                    nc.vector.scalar_tensor_tensor(
                        v2[:, cs], ref[:, cs], 0.0, v1[:, cs],
                        op.is_le, op.mult)
                    nc.vector.tensor_scalar(
                        spk_h[:], v2[:, cs], V_TH, None, op.is_ge)
                    nc.scalar.copy(spk_f[:, cs], spk_h[:])
                    for cq in range(4):
                        tq = pst.tile([128, 192], bf16, tag="tp")
                        nc.tensor.transpose(
                            tq[:, 0:128],
                            spk_h[:, 128 * cq:128 * cq + 128],
                            i128b[:])
                        s0 = 32 * d + 2 * cq + 1
                        dst = YIs[:, s0:s0 + 25:8, :]
                        src_ = tq[:, 0:128].rearrange("p (g c) -> p g c",
                                                      c=32)
                        nc.scalar.copy(dst, src_)
                    if d == 0:
                        nc.vector.tensor_copy(YIs[0:64, 2:33:2, :],
                                              YIs[64:128, 1:32:2, :])
                        nc.vector.tensor_copy(YIs[64:128, 0:32:2, :],
                                              YIs[0:64, 1:33:2, :])
                        nc.vector.scalar_tensor_tensor(
                            YIp[tb][:, 0:33, :], YIp[nb][:, 0:33, :],
                            BETA_POST, YIs[:, 0:33, :], op.mult, op.add)
                    else:
                        nc.vector.tensor_copy(YIs[0:64, 34:65:2, :],
                                              YIs[64:128, 33:64:2, :])
                        nc.vector.tensor_copy(YIs[64:128, 32:64:2, :],
                                              YIs[0:64, 33:65:2, :])
                        nc.vector.scalar_tensor_tensor(
                            YIp[tb][:, 33:65, :], YIp[nb][:, 33:65, :],
                            BETA_POST, YIs[:, 33:65, :], op.mult, op.add)

                # ---- 8: refresh stale weights for next step ----
                nc.vector.scalar_tensor_tensor(
                    Wrs[0:48, :], Wk[:], 1.0, zero96[:], op.mult, op.add)
                nc.vector.scalar_tensor_tensor(
                    Wrs[64:112, :], Wk[:], 1.0, zero96[:], op.mult, op.add)
                nc.scalar.dma_start(WLr[48:96, :], Wrs[0:48, :])
                # tail
                nc.vector.scalar_tensor_tensor(
                    v[:], v2[:], V_TH, v2[:], op.is_lt, op.mult)
                rrelu = io.tile([128, 1024], f32, tag="rrelu")
                nc.scalar.activation(
                    rrelu[:], ref[:], mybir.ActivationFunctionType.Relu,
                    bias=negone[:], scale=1.0)
                nc.vector.scalar_tensor_tensor(
                    ref[:], spk_f[:], T_REF, rrelu[:], op.mult, op.add)
                spk8 = io.tile([128, 1024], u8, tag="spk8")
                nc.scalar.copy(spk8[:], spk_f[:])



                # ---- 15: dW matmuls ----
                dps = psw.tile([128, 96], f32, tag="dps")
                for k in range(32):
                    nc.tensor.matmul(dps[0:48, :], P3[tb][:, k, :],
                                     YIs[:, 2 * k:2 * k + 3, :],
                                     start=(k == 0), stop=(k == 31))
                for k in range(32):
                    nc.tensor.matmul(dps[64:112, :], Sl[t % 3][:, k, :],
                                     YIp[tb][:, 2 * k:2 * k + 3, :],
                                     start=(k == 0), stop=(k == 31),
                                     tile_position=(0, 64))

                # ---- 16: pack + AllGather ----
                if t < n_steps - 1:
                    ccs = io.tile([48, 192], bf16, tag="ccs")
                    nc.scalar.copy(ccs[:, 0:96], dps[0:48, :])
                    nc.scalar.copy(ccs[:, 96:192], dps[64:112, :])
                    nc.gpsimd.dma_start(cc_in[t][:], ccs[:])
                    nc.gpsimd.collective_compute(
                        "AllGather", op.bypass,
                        replica_groups=[list(range(N_CORES))],
                        ins=[cc_in[t].ap().opt()],
                        outs=[cc_out[t].ap().opt()],
                    )

                # ---- outputs issued after the AG trigger so the DMA
                # queues stay clear for the collective's ring traffic ----
                nc.sync.dma_start(spk_d[t], spk8[:])
                nc.sync.dma_start(v_d[t], v[:])
                nc.sync.dma_start(i_d[t], isyn2[:])

    _split_sync_waits(nc)
    return nc


def _prep_inputs(S, W0):
    """Host-side layout transforms (pure data movement, no math)."""
    import ml_dtypes

    bf = ml_dtypes.bfloat16
    Tn = S.shape[0]
    # Sc: [T, 112, 2176] kw-shifted S3c layout (rows 48-63 unused pad)
    Sc = np.zeros((Tn, 112, 34, 64), np.float32)
    for h in (0, 1):
        r0 = 64 * h
        hr0 = 0 if h == 0 else 31
        jc0 = 1 if h == 0 else 0
        blk = S[:, :, hr0:hr0 + 33, :]  # [T, 16, 33, 64]
        Sc[:, r0 + 16:r0 + 32, jc0:jc0 + 33, :] = blk
        Sc[:, r0 + 0:r0 + 16, jc0:jc0 + 33, 1:64] = blk[..., 0:63]
        Sc[:, r0 + 32:r0 + 48, jc0:jc0 + 33, 0:63] = blk[..., 1:64]
    Sc = Sc.reshape(Tn, 112, XW)
    # Sl: [T, 128, 32, 48]; Sl[t, p, 16h+b, j] = Sc[t, 64h+j, 64+128b+p]
    Sl = np.zeros((Tn, 128, 32, 48), np.float32)
    for h in (0, 1):
        blk = Sc[:, 64 * h:64 * h + 48, 64:64 + 2048]
        blk = blk.reshape(Tn, 48, 16, 128)
        Sl[:, :, 16 * h:16 * h + 16, :] = blk.transpose(0, 3, 2, 1)
    return (np.ascontiguousarray(Sc.astype(bf)),
            np.ascontiguousarray(Sl.astype(bf)))


def kernel(S, W0):
    from concourse import bass_utils

    S = np.ascontiguousarray(np.asarray(S, np.float32))
    W0 = np.asarray(W0, np.float32)
    Wk = np.ascontiguousarray(
        W0[:, :, ::-1, :].transpose(3, 1, 2, 0).reshape(48, 96))

    key = T
    if key not in _NC_CACHE:
        _NC_CACHE[key] = _build(T)
    nc = _NC_CACHE[key]

    in_maps = []
    for r in range(N_CORES):
        Sc, Sl = _prep_inputs(S[:, r], W0)
        in_maps.append({"Sc": Sc, "Sl": Sl, "Wk": Wk})
    res = bass_utils.run_bass_kernel_spmd(nc, in_maps, core_ids=list(range(N_CORES)))
    global LAST_EXEC_NS, LAST_RES
    LAST_EXEC_NS = getattr(res, "exec_time_ns", None)
    LAST_RES = res

    spikes = np.zeros((T, B, C_OUT, H, W_IN), np.bool_)
    v_traj = np.zeros((T, B, C_OUT, H, W_IN), np.float32)
    i_traj = np.zeros((T, B, C_OUT, H, W_IN), np.float32)

    def unfold(a):
        # [T,128,1024] fold (p=32g+o, col=512d+r) -> [T, 32, 64, 64]
        a = np.asarray(a).reshape(T, 4, 32, 2, 512)
        return a.transpose(0, 2, 3, 1, 4).reshape(T, C_OUT, H, W_IN)

    for r in range(N_CORES):
        o = res.results[r]
        spikes[:, r] = unfold(o["spk_out"]) != 0
        v_traj[:, r] = unfold(o["v_out"])
        i_traj[:, r] = unfold(o["i_out"])
    return spikes, v_traj, i_traj


# revision 24
# speedup vs baseline: 1.0005x; 1.0005x over previous
"""LIF Conv2d + STDP kernel for 8 Trainium2 NeuronCores — v3.

Data-parallel over batch (B=8, one element per core); per-step STDP
weight-gradient correlations AllGathered (bf16) and summed locally.

Changes vs v2:
  - Host-precomputed input layouts: Sc [T,112,2176] bf16 (the S3c
    kw-shifted layout, DMA'd straight in with large bursts) and
    Sl [T,128,32,48] bf16 (l-major S patches), killing the 32 per-step
    PE transposes and the S3cb cast.
  - Outputs written in the native fold layout [T,128,1024] (one DMA
    per tensor per step, 1-4KB bursts); unscrambled on host.
  - Triple-buffered Sc/Sl with 2-step prefetch; double-buffered
    xr/XHf/x3f so the x-path for step t+1 runs during step t's
    AllGather window.
  - The f32r xl correction pass and the Wfix fix-up pass are merged
    into ONE f32r late pass: lhsT = [Wfix ; Wrs_stale] (96 rows), rhs
    XHf = [xr ; xl] stacked per l-half (filled via SBUF-SBUF DMA since
    engine APs must start at partition 0/32/64/96).  24 early + 24
    late f32r matmuls (was 72), telescoping to W_new@x exactly like
    the baseline's three passes.
  - DMA issue spread across engines: prefetch/outputs on sync, x/W
    staging moves on scalar, dw8/cc_in/collective on gpsimd.

Remaining bottleneck: the per-step AllGather round-trip chain
(pack -> trigger ~2.5us -> AG ~7.5us -> dw8 load ~3.8us -> DVE W chain
~3.7us) stalls the PE ~11-19us/step; early conv of t+1 is the only
AG-independent PE work and fills ~8us.  cc payload is column-packed
[48,192] (dWp|dWm) so the consumer needs no cross-partition copy;
dw8 load issues from sync; output DMAs issue after the AG trigger to
keep queues clear for the ring.  cc_in issue must stay on gpsimd
(moving it to sync measured slower).  2-deep pipelining does NOT help:
there is exactly one early-conv block per step to fill each stall.

Known constraints found the hard way: f32r matmuls may only write
PSUM at partition base 0 (bf16 can use 64); tensor_tensor requires
matching start partitions across out/in APs (stt only across inputs);
f32r streams at ~0.83 ns/col regardless of grouping or p-state games
(PE warmer matmuls were tried twice and always run at the 1.2GHz
rate, costing more than they save); moving the Wrs refresh after the
LIF in DVE program order delays the next step's early conv.
"""

import numpy as np

T, B, C_IN, H, W_IN = 32, 8, 16, 64, 64
C_OUT, KH, KW = 32, 3, 3
L = H * W_IN  # 4096
XW = 2176  # 2048 + 2*64 halo
BETA_M = float(np.exp(-1.0 / 20.0))
BETA_S = float(np.exp(-1.0 / 5.0))
BETA_PRE = float(np.exp(-1.0 / 20.0))
BETA_POST = float(np.exp(-1.0 / 20.0))
V_TH = 1.0
T_REF = 2.0
ETA = 5e-4
NORM = float(B * L)
N_CORES = 8


def _patch_tile_drain():
    """walrus in this build rejects >1 sync wait on a CTRL-class (drain)
    instruction; spread the final tile drain's waits across nops."""
    import concourse.tile as tile
    import concourse.mybir as mybir
    from concourse.vector_clock import ScopedClock

    if getattr(tile.TileContext, "_drain_patched", False):
        return

    def _drain_and_barrier(self, tick_clock, wait_clock):
        nc = self.nc
        drain_inst = nc.sync.drain()
        wait_clock.add_sem_waits(
            drain_inst.ins, ScopedClock({None: tick_clock.global_clock})
        )
        si = drain_inst.ins.sync_info
        waits = list(si.on_wait or [])
        if len(waits) > 1:
            si.on_wait = waits[:1]
            for i in range(1, len(waits)):
                nop = nc.sync.nop(nofuse=True)
                nop.ins.sync_info = mybir.SyncInfo(
                    on_wait=waits[i : i + 1], on_update=[]
                )
        nc.all_engine_barrier()
        assert self.sems is not None
        popped = nc._tile_sem_poison_stack.pop()
        assert popped is self._sem_poison
        nc.clear_and_free_semaphores(list(self.sems.allocated().values()))
        nc.all_engine_barrier()

    tile.TileContext._drain_and_barrier = _drain_and_barrier
    tile.TileContext._drain_patched = True


def _split_sync_waits(nc):
    """This walrus build accepts only ONE sync-wait slot per instruction.
    Move extra waits onto injected same-engine nops placed just before."""
    import concourse.mybir as mybir

    n = 0
    for f in nc.m.functions:
        for bb in f.blocks:
            new_insts = []
            for inst in bb.instructions:
                si = inst.sync_info
                waits = list(si.on_wait or []) if si else []
                if len(waits) > 1:
                    for w in waits[:-1]:
                        n += 1
                        nop = mybir.InstNoOp(
                            name=f"I-wsplit-{n}", engine=inst.engine,
                            ins=[], outs=[], bass_nofuse=True,
                            sync_info=mybir.SyncInfo(on_wait=[w], on_update=[]),
                        )
                        new_insts.append(nop)
                    si.on_wait = waits[-1:]
                new_insts.append(inst)
            bb.instructions = new_insts
    return n


_NC_CACHE = {}


def _build(n_steps):
    import concourse.bass as bass
    import concourse.mybir as mybir
    import concourse.tile as tile

    _patch_tile_drain()
    f32 = mybir.dt.float32
    f32r = mybir.dt.float32r
    bf16 = mybir.dt.bfloat16
    u8 = mybir.dt.uint8
    op = mybir.AluOpType

    nc = bass.Bass("TRN2", target_bir_lowering=False, debug=False,
                   num_devices=N_CORES)

    Sc_d = nc.dram_tensor("Sc", [n_steps, 112, XW], bf16, kind="ExternalInput")
    Sl_d = nc.dram_tensor("Sl", [n_steps, 128, 32, 48], bf16,
                          kind="ExternalInput")
    W_d = nc.dram_tensor("Wk", [48, 96], f32, kind="ExternalInput")
    spk_d = nc.dram_tensor("spk_out", [n_steps, 128, 1024], u8,
                           kind="ExternalOutput")
    v_d = nc.dram_tensor("v_out", [n_steps, 128, 1024], f32,
                         kind="ExternalOutput")
    i_d = nc.dram_tensor("i_out", [n_steps, 128, 1024], f32,
                         kind="ExternalOutput")

    cc_in = [nc.dram_tensor(f"cc_in_{t}", [48, 192], bf16)
             for t in range(n_steps - 1)]
    cc_out = [
        nc.dram_tensor(f"cc_out_{t}", [N_CORES, 48, 192], bf16,
                       addr_space="Shared")
        for t in range(n_steps - 1)
    ]

    id128 = nc.inline_tensor(np.eye(128, dtype=np.float32), "id128")

    eta_n = ETA / NORM

    with tile.TileContext(nc) as tc:
        with (
            tc.tile_pool(name="state", bufs=1) as st,
            tc.tile_pool(name="io", bufs=2) as io,
            tc.tile_pool(name="psc", bufs=4, space=bass.MemorySpace.PSUM) as psc,
            tc.tile_pool(name="pst", bufs=3, space=bass.MemorySpace.PSUM) as pst,
            tc.tile_pool(name="psw", bufs=1, space=bass.MemorySpace.PSUM) as psw,
        ):
            # ---------------- persistent state ----------------
            x3f = [st.tile([112, XW], f32, tag=f"x3f{i}", name=f"x3f{i}") for i in range(2)]
            xr = [st.tile([112, XW], f32r, tag=f"xr{i}", name=f"xr{i}") for i in range(2)]
            # XHf[h][buf] (f32r): rows 0-47 = xr, rows 48-95 = xl for
            # l-half h.  WLr rows 0-47 = Wfix (engine-written, critical),
            # rows 48-95 = Wrs_stale (DMA'd off-critical).  The late pass
            # [Wfix;Wrs] @ [xr;xl] telescopes to W_new@x exactly like the
            # baseline's two separate correction passes.
            XHf = [[st.tile([96, XW], f32r, tag=f"XHf{h}{i}", name=f"XHf{h}{i}")
                    for i in range(2)] for h in range(2)]
            Sc = [st.tile([112, XW], bf16, tag=f"Sc{i}", name=f"Sct{i}") for i in range(3)]
            Sl = [st.tile([128, 32, 48], bf16, tag=f"Sl{i}", name=f"Slt{i}") for i in range(3)]
            Wk = st.tile([48, 96], f32, tag="Wk")
            Wrs = st.tile([112, 96], f32r, tag="Wrs")
            WLr = st.tile([96, 96], f32r, tag="WLr")
            wtmp = st.tile([48, 96], f32, tag="wtmp")
            wtmp2 = st.tile([48, 96], f32, tag="wtmp2")
            zero96 = st.tile([48, 96], f32, tag="zero96")
            v = st.tile([128, 1024], f32, tag="v")
            ref = st.tile([128, 1024], f32, tag="ref")
            isyn2 = st.tile([128, 1024], f32, tag="isyn2")
            isyn = st.tile([128, 1024], f32, tag="isyn")
            xlf = st.tile([112, XW], f32r, tag="xlf")
            P3 = [st.tile([128, 32, 48], bf16, tag=f"P3{i}", name=f"P3t{i}") for i in range(2)]
            YIs = st.tile([128, 65, 32], bf16, tag="YIs")
            YIp = [st.tile([128, 65, 32], bf16, tag=f"YIp{i}", name=f"YIpt{i}") for i in range(2)]
            i128b = st.tile([128, 128], bf16, tag="i128b")
            negone = st.tile([128, 1], f32, tag="negone")
            i128f = st.tile([128, 128], f32, tag="i128f")

            # ---------------- prologue ----------------
            nc.sync.dma_start(Wk[:], W_d[:])
            nc.sync.dma_start(i128f[:], id128[:])
            nc.vector.tensor_copy(i128b[:], i128f[:])
            nc.sync.dma_start(Sc[0][:], Sc_d[0])
            nc.sync.dma_start(Sl[0][:], Sl_d[0])
            nc.sync.dma_start(Sc[1][:], Sc_d[1])
            nc.sync.dma_start(Sl[1][:], Sl_d[1])
            nc.vector.memset(x3f[1][:], 0.0)
            nc.vector.memset(zero96[:], 0.0)
            nc.vector.memset(v[:], 0.0)
            nc.vector.memset(ref[:], 0.0)
            nc.vector.memset(P3[1][:], 0.0)
            nc.vector.memset(YIs[:], 0.0)
            nc.vector.memset(YIp[1][:], 0.0)
            nc.vector.memset(negone[:], -1.0)
            nc.vector.scalar_tensor_tensor(
                Wrs[0:48, :], Wk[:], 1.0, zero96[:], op.mult, op.add)
            nc.vector.scalar_tensor_tensor(
                Wrs[64:112, :], Wk[:], 1.0, zero96[:], op.mult, op.add)
            nc.scalar.dma_start(WLr[48:96, :], Wrs[0:48, :])
            # x-path for t=0
            nc.vector.scalar_tensor_tensor(
                x3f[0][:], x3f[1][:], BETA_S, Sc[0][:], op.mult, op.add)
            nc.vector.scalar_tensor_tensor(
                xr[0][:], x3f[1][:], BETA_S, Sc[0][:], op.mult, op.add)
            nc.vector.tensor_tensor(xlf[:], x3f[0][:], xr[0][:], op.subtract)
            for h in range(2):
                r0 = 64 * h
                nc.scalar.dma_start(XHf[h][0][0:48, :], xr[0][r0:r0 + 48, :])
                nc.scalar.dma_start(XHf[h][0][48:96, :], xlf[r0:r0 + 48, :])
            # P3 for t=0
            nc.vector.scalar_tensor_tensor(
                P3[0][:], P3[1][:], BETA_PRE, Sl[0][:], op.mult, op.add)

            for t in range(n_steps):
                tb = t % 2
                nb = (t + 1) % 2
                # ---- 1: prefetch S layouts two steps ahead ----
                if t + 2 < n_steps:
                    nc.sync.dma_start(Sc[(t + 2) % 3][:], Sc_d[t + 2])
                    nc.sync.dma_start(Sl[(t + 2) % 3][:], Sl_d[t + 2])

                # ---- 2: EARLY conv (f32r): isyn = Wrs @ xr ----
                for j in range(8):
                    h, jq = j // 4, j % 4
                    g, d = j % 4, j // 4
                    pe = psc.tile([32, 512], f32, tag="pe")
                    for kh in (1, 0, 2):
                        c0 = 512 * jq + 64 * kh
                        khr = 2 - kh
                        nc.tensor.matmul(
                            pe[:],
                            Wrs[64 * h:64 * h + 48, 32 * khr:32 * khr + 32],
                            xr[tb][64 * h:64 * h + 48, c0:c0 + 512],
                            start=(kh == 1), stop=(kh == 2))
                    sl = isyn[32 * g:32 * g + 32, 512 * d:512 * d + 512]
                    if d == 0:
                        nc.vector.tensor_copy(sl, pe[:])
                    else:
                        nc.scalar.copy(sl, pe[:])

                # ---- 3/4: x-path + P3 for t+1 (fills the AG window) ----
                if t + 1 < n_steps:
                    scn = Sc[(t + 1) % 3]
                    nc.vector.scalar_tensor_tensor(
                        x3f[nb][:], x3f[tb][:], BETA_S, scn[:], op.mult, op.add)
                    nc.vector.scalar_tensor_tensor(
                        xr[nb][:], x3f[tb][:], BETA_S, scn[:], op.mult, op.add)
                    nc.vector.tensor_tensor(xlf[:], x3f[nb][:], xr[nb][:],
                                            op.subtract)
                    for h in range(2):
                        r0 = 64 * h
                        nc.scalar.dma_start(XHf[h][nb][0:48, :],
                                            xr[nb][r0:r0 + 48, :])
                        nc.scalar.dma_start(XHf[h][nb][48:96, :],
                                            xlf[r0:r0 + 48, :])
                    nc.vector.scalar_tensor_tensor(
                        P3[nb][:], P3[tb][:], BETA_PRE, Sl[(t + 1) % 3][:],
                        op.mult, op.add)

                # ---- 5: weight update from AG(t-1) ----
                if t > 0:
                    dw8 = io.tile([48, N_CORES, 192], bf16, tag="dw8")
                    gv = cc_out[t - 1].ap().rearrange("r p c -> p r c")
                    nc.sync.dma_start(dw8[:], gv)
                    s4 = io.tile([48, 4, 192], bf16, tag="s4")
                    nc.vector.tensor_tensor(
                        s4[:], dw8[:, 0:4, :], dw8[:, 4:8, :], op.add)
                    s2 = io.tile([48, 2, 192], bf16, tag="s2")
                    nc.vector.tensor_tensor(
                        s2[:], s4[:, 0:2, :], s4[:, 2:4, :], op.add)
                    dws = io.tile([48, 192], f32, tag="dws")
                    nc.vector.tensor_tensor(
                        dws[:], s2[:, 0, :], s2[:, 1, :], op.add)
                    # fused update: W = W*(1 - eta*(dWp+dWm)) + eta*dWp; clip
                    nc.vector.tensor_tensor(
                        wtmp[:], dws[:, 0:96], dws[:, 96:192], op.add)
                    nc.vector.tensor_scalar(wtmp[:], wtmp[:], -eta_n, 1.0,
                                            op.mult, op.add)
                    nc.vector.tensor_tensor(wtmp2[:], Wk[:], wtmp[:], op.mult)
                    nc.vector.scalar_tensor_tensor(
                        Wk[:], dws[:, 0:96], eta_n, wtmp2[:], op.mult, op.add)
                    nc.vector.tensor_scalar(Wk[:], Wk[:], 1.0, 0.0,
                                            op.min, op.max)

                # ---- 6: Wfix = W_new - Wrs_stale (f32r) ----
                nc.vector.tensor_tensor(wtmp[:], Wk[:], Wrs[0:48, :],
                                        op.subtract)
                nc.vector.scalar_tensor_tensor(
                    WLr[0:48, :], wtmp[:], 1.0, zero96[:], op.mult, op.add)

                # ---- 7: LATE conv + LIF half-interleaved: LIF for half
                # d runs on the DVE while the other half's matmuls stream --
                v1 = io.tile([128, 1024], f32, tag="v1")
                v2 = io.tile([128, 1024], f32, tag="v2")
                spk_bA = io.tile([128, 512], bf16, tag="spk_bA")
                spk_bB = io.tile([128, 512], bf16, tag="spk_bB")
                spk_f = io.tile([128, 1024], f32, tag="spk_f")
                for d in range(2):
                    h = d
                    for jq in range(4):
                        g = jq
                        pl = psc.tile([32, 512], f32, tag="pe", name="pl")
                        for kh in (1, 0, 2):
                            c0 = 512 * jq + 64 * kh
                            khr = 2 - kh
                            nc.tensor.matmul(
                                pl[:],
                                WLr[0:96, 32 * khr:32 * khr + 32],
                                XHf[h][tb][0:96, c0:c0 + 512],
                                start=(kh == 1), stop=(kh == 2))
                        sl = isyn[32 * g:32 * g + 32,
                                  512 * d:512 * d + 512]
                        sl2 = isyn2[32 * g:32 * g + 32,
                                    512 * d:512 * d + 512]
                        nc.vector.tensor_tensor(sl2, sl, pl[:], op.add)
                    spk_h = spk_bA if d == 0 else spk_bB
                    cs = slice(512 * d, 512 * d + 512)
                    nc.vector.scalar_tensor_tensor(
                        v1[:, cs], v[:, cs], BETA_M, isyn2[:, cs],
                        op.mult, op.add)
                    nc.vector.scalar_tensor_tensor(
                        v2[:, cs], ref[:, cs], 0.0, v1[:, cs],
                        op.is_le, op.mult)
                    nc.vector.tensor_scalar(
                        spk_h[:], v2[:, cs], V_TH, None, op.is_ge)
                    nc.scalar.copy(spk_f[:, cs], spk_h[:])
                    for cq in range(4):
                        tq = pst.tile([128, 192], bf16, tag="tp")
                        nc.tensor.transpose(
                            tq[:, 0:128],
                            spk_h[:, 128 * cq:128 * cq + 128],
                            i128b[:])
                        s0 = 32 * d + 2 * cq + 1
                        dst = YIs[:, s0:s0 + 25:8, :]
                        src_ = tq[:, 0:128].rearrange("p (g c) -> p g c",
                                                      c=32)
                        nc.scalar.copy(dst, src_)
                    if d == 0:
                        nc.vector.tensor_copy(YIs[0:64, 2:33:2, :],
                                              YIs[64:128, 1:32:2, :])
                        nc.vector.tensor_copy(YIs[64:128, 0:32:2, :],
                                              YIs[0:64, 1:33:2, :])
                        nc.vector.scalar_tensor_tensor(
                            YIp[tb][:, 0:33, :], YIp[nb][:, 0:33, :],
                            BETA_POST, YIs[:, 0:33, :], op.mult, op.add)
                    else:
                        nc.vector.tensor_copy(YIs[0:64, 34:65:2, :],
                                              YIs[64:128, 33:64:2, :])
                        nc.vector.tensor_copy(YIs[64:128, 32:64:2, :],
                                              YIs[0:64, 33:65:2, :])
                        nc.vector.scalar_tensor_tensor(
                            YIp[tb][:, 33:65, :], YIp[nb][:, 33:65, :],
                            BETA_POST, YIs[:, 33:65, :], op.mult, op.add)

                # ---- 8: refresh stale weights for next step ----
                nc.vector.scalar_tensor_tensor(
                    Wrs[0:48, :], Wk[:], 1.0, zero96[:], op.mult, op.add)
                nc.vector.scalar_tensor_tensor(
                    Wrs[64:112, :], Wk[:], 1.0, zero96[:], op.mult, op.add)
                nc.scalar.dma_start(WLr[48:96, :], Wrs[0:48, :])
                # tail
                nc.vector.scalar_tensor_tensor(
                    v[:], v2[:], V_TH, v2[:], op.is_lt, op.mult)
                rrelu = io.tile([128, 1024], f32, tag="rrelu")
                nc.scalar.activation(
                    rrelu[:], ref[:], mybir.ActivationFunctionType.Relu,
                    bias=negone[:], scale=1.0)
                nc.vector.scalar_tensor_tensor(
                    ref[:], spk_f[:], T_REF, rrelu[:], op.mult, op.add)
                spk8 = io.tile([128, 1024], u8, tag="spk8")
                nc.scalar.copy(spk8[:], spk_f[:])



                # ---- 15: dW matmuls ----
                dps = psw.tile([128, 96], f32, tag="dps")
                for k in range(32):
                    nc.tensor.matmul(dps[0:48, :], P3[tb][:, k, :],
                                     YIs[:, 2 * k:2 * k + 3, :],
                                     start=(k == 0), stop=(k == 31))
                for k in range(32):
                    nc.tensor.matmul(dps[64:112, :], Sl[t % 3][:, k, :],
                                     YIp[tb][:, 2 * k:2 * k + 3, :],
                                     start=(k == 0), stop=(k == 31),
                                     tile_position=(0, 64))

                # ---- 16: pack + AllGather ----
                if t < n_steps - 1:
                    ccs = io.tile([48, 192], bf16, tag="ccs")
                    nc.scalar.copy(ccs[:, 0:96], dps[0:48, :])
                    nc.scalar.copy(ccs[:, 96:192], dps[64:112, :])
                    nc.gpsimd.dma_start(cc_in[t][:], ccs[:])
                    nc.gpsimd.collective_compute(
                        "AllGather", op.bypass,
                        replica_groups=[list(range(N_CORES))],
                        ins=[cc_in[t].ap().opt()],
                        outs=[cc_out[t].ap().opt()],
                    )

                # ---- outputs issued after the AG trigger so the DMA
                # queues stay clear for the collective's ring traffic ----
                nc.sync.dma_start(spk_d[t], spk8[:])
                nc.sync.dma_start(v_d[t], v[:])
                nc.sync.dma_start(i_d[t], isyn2[:])

    _split_sync_waits(nc)
    return nc


def _prep_inputs(S, W0):
    """Host-side layout transforms (pure data movement, no math)."""
    import ml_dtypes

    bf = ml_dtypes.bfloat16
    Tn = S.shape[0]
    # Sc: [T, 112, 2176] kw-shifted S3c layout (rows 48-63 unused pad)
    Sc = np.zeros((Tn, 112, 34, 64), np.float32)
    for h in (0, 1):
        r0 = 64 * h
        hr0 = 0 if h == 0 else 31
        jc0 = 1 if h == 0 else 0
        blk = S[:, :, hr0:hr0 + 33, :]  # [T, 16, 33, 64]
        Sc[:, r0 + 16:r0 + 32, jc0:jc0 + 33, :] = blk
        Sc[:, r0 + 0:r0 + 16, jc0:jc0 + 33, 1:64] = blk[..., 0:63]
        Sc[:, r0 + 32:r0 + 48, jc0:jc0 + 33, 0:63] = blk[..., 1:64]
    Sc = Sc.reshape(Tn, 112, XW)
    # Sl: [T, 128, 32, 48]; Sl[t, p, 16h+b, j] = Sc[t, 64h+j, 64+128b+p]
    Sl = np.zeros((Tn, 128, 32, 48), np.float32)
    for h in (0, 1):
        blk = Sc[:, 64 * h:64 * h + 48, 64:64 + 2048]
        blk = blk.reshape(Tn, 48, 16, 128)
        Sl[:, :, 16 * h:16 * h + 16, :] = blk.transpose(0, 3, 2, 1)
    return (np.ascontiguousarray(Sc.astype(bf)),
            np.ascontiguousarray(Sl.astype(bf)))


def kernel(S, W0):
    from concourse import bass_utils

    S = np.ascontiguousarray(np.asarray(S, np.float32))
    W0 = np.asarray(W0, np.float32)
    Wk = np.ascontiguousarray(
        W0[:, :, ::-1, :].transpose(3, 1, 2, 0).reshape(48, 96))

    key = T
    if key not in _NC_CACHE:
        _NC_CACHE[key] = _build(T)
    nc = _NC_CACHE[key]

    in_maps = []
    for r in range(N_CORES):
        Sc, Sl = _prep_inputs(S[:, r], W0)
        in_maps.append({"Sc": Sc, "Sl": Sl, "Wk": Wk})
    res = bass_utils.run_bass_kernel_spmd(nc, in_maps, core_ids=list(range(N_CORES)))
    global LAST_EXEC_NS, LAST_RES
    LAST_EXEC_NS = getattr(res, "exec_time_ns", None)
    LAST_RES = res

    spikes = np.zeros((T, B, C_OUT, H, W_IN), np.bool_)
    v_traj = np.zeros((T, B, C_OUT, H, W_IN), np.float32)
    i_traj = np.zeros((T, B, C_OUT, H, W_IN), np.float32)

    def unfold(a):
        # [T,128,1024] fold (p=32g+o, col=512d+r) -> [T, 32, 64, 64]
        a = np.asarray(a).reshape(T, 4, 32, 2, 512)
        return a.transpose(0, 2, 3, 1, 4).reshape(T, C_OUT, H, W_IN)

    for r in range(N_CORES):
        o = res.results[r]
        spikes[:, r] = unfold(o["spk_out"]) != 0
        v_traj[:, r] = unfold(o["v_out"])
        i_traj[:, r] = unfold(o["i_out"])
    return spikes, v_traj, i_traj
